# revision 20
# baseline (speedup 1.0000x reference)
"""Trainium2 Bass kernel for nn_LocalRouter (sparse_attention).

Computation (reference semantics):
  local:  h_w = silu(mu_n @ Wm1_top + mu_{n-w} @ Wm1_bot + bm1), w=1..4
          local = mean_w(h_w) @ Wm2 + bm2
  global: scores = (mu @ Wq) @ (mu @ Wk)^T / sqrt(D), causal; top-8 -> softmax
          global = probs @ mu @ Wv + bv        (rows of probs sum to 1)
  out = concat([local, global]) @ Wo + bo

Algebraic refactors (host-side, exact in fp32):
  scores = (mu @ Wqks) @ mu^T, Wqks = Wq @ Wk^T / sqrt(D)
  out[n] = (hbar @ Wmo)[n] + sum_k p_k V[idx_k]
    hbar = sum_w silu(...), Wmo = (Wm2 @ Wo_top)/4
    V = mu @ (Wv @ Wo_bot) + bconst   (bconst folds in exactly: sum_k p_k = 1)
    bconst = bo + bm2 @ Wo_top + bv @ Wo_bot

Precision: scores need ~2^-20 accuracy (top-8 boundary flips swap whole V
rows). Computed as a 3-pass fp16 hi/lo split (1 cyc/row on the PE):
  mu = mh + ml_s/256,  qh = qhh + qhl          (fp16 pairs; lo parts scaled
  Wqks = Wh + Wl_s/256                          into fp16-normal range)
  qh     = mh@Wh + ml_s@(Wh/256) + (mh/256)@Wl_s      [fp32 PSUM accum]
  scores = qhh@mh + qhl@mh + (qhh/256)@ml_s

Engine plan (per core; v2 restructure of the 625us baseline):
  - qh computed per GROUP of 4 slots at FD=512 (was per-slot FD=128): the
    FD=128 matmuls were LDWEIGHTS-bound (~82us -> ~41us).
  - top-8 weighted sum moved off the DVE (was ~112us of 1x-mode tensor_scalar
    with per-partition scalars) onto the PE as 8 diag(p8_k) @ g_k matmuls
    accumulated into the same PSUM bank as the out-row projection.  diag
    matrices are built on the Scalar engine (activation Copy, per-partition
    scale AP reading eye(128)).
  - softmax exp replaced with tanh: e^x = sig/(1-sig), sig = (1+tanh(x/2))/2.
    Tanh lives in the same ACT table set as Silu, so the per-slot
    ACT_TABLE_LOAD churn (17 loads, ~45us scalar) disappears.
  - group g+1's DMA loads + qh + local branch are PREFETCHED at slot 4g+3 so
    the PE never waits on qh tiles at group boundaries.
  - out rows written f16 (host upcasts).

Top-8 indices are rewrapped for dma_gather on-chip with tiny selection
matmuls (iw[p, c*8+pb] = i8[16*pb + p%16, c]).

Sharding: core c -> batch b=c//2, half h=c%2 owns query tiles {t: t%2==h}
(interleaved for causal load balance). Key range per slot s is 256*(s+1)
(h-independent; the h-dependent diagonal lives in the trimask data).
"""

import math
import numpy as np

B, N, D = 4, 4096, 512
WIN, TOPK = 4, 8
P = 128
NCORES = 8
NSLOT = 16            # query tiles owned per core
NEG = -1.0e30
ASC = 256.0           # hi/lo split scale (2^8)

_cache = {}


def _build_program():
    if "nc" in _cache:
        return _cache["nc"]
    from contextlib import ExitStack
    import concourse.bass as bass
    import concourse.tile as tile
    import concourse.mybir as mybir
    from concourse import bacc

    dt = mybir.dt
    AF = mybir.ActivationFunctionType
    OP = mybir.AluOpType

    nc = bacc.Bacc(
        "TRN2",
        target_bir_lowering=False,
        debug=False,
        enable_asserts=False,
        num_devices=NCORES,
    )

    f32, f16 = dt.float32, dt.float16
    # ---- DRAM I/O (per-core data; program identical on all cores) ----
    muT_h = nc.dram_tensor("muT_h", [4, P, 4 + N], f16, kind="ExternalInput").ap()
    muT_ls = nc.dram_tensor("muT_ls", [4, P, N], f16, kind="ExternalInput").ap()
    muloc_h = nc.dram_tensor("muloc_h", [4, P, NSLOT * 132], f16,
                             kind="ExternalInput").ap()
    muloc_ls = nc.dram_tensor("muloc_ls", [4, P, NSLOT * 132], f16,
                              kind="ExternalInput").ap()
    vkeys = nc.dram_tensor("vkeys", [N, D], f16, kind="ExternalInput").ap()
    whh = nc.dram_tensor("whh", [4, P, D], f16, kind="ExternalInput").ap()
    whd = nc.dram_tensor("whd", [4, P, D], f16, kind="ExternalInput").ap()
    wls = nc.dram_tensor("wls", [4, P, D], f16, kind="ExternalInput").ap()
    wm1t = nc.dram_tensor("wm1t", [4, P, D], f16, kind="ExternalInput").ap()
    wm1b = nc.dram_tensor("wm1b", [4, P, D], f16, kind="ExternalInput").ap()
    wmo = nc.dram_tensor("wmo", [4, P, D], f16, kind="ExternalInput").ap()
    trimask = nc.dram_tensor("trimask", [P, 256], f32, kind="ExternalInput").ap()
    bm1t = nc.dram_tensor("bm1t", [P, 4], f32, kind="ExternalInput").ap()
    e8sel = nc.dram_tensor("e8sel", [P, P], f32, kind="ExternalInput").ap()
    repm = nc.dram_tensor("repm", [16, P], f32, kind="ExternalInput").ap()
    outR = nc.dram_tensor("outR", [NSLOT * P, D], f16, kind="ExternalOutput").ap()

    with tile.TileContext(nc) as tc, ExitStack() as ctx:
        consts = ctx.enter_context(tc.tile_pool(name="consts", bufs=1))
        qh_pool = ctx.enter_context(tc.tile_pool(name="qh", bufs=1))
        mhd_pool = ctx.enter_context(tc.tile_pool(name="mhd", bufs=1))
        strip_pool = ctx.enter_context(tc.tile_pool(name="strip", bufs=2))
        top_pool = ctx.enter_context(tc.tile_pool(name="top", bufs=2))
        diag_pool = ctx.enter_context(tc.tile_pool(name="diag", bufs=2))
        g_pool = ctx.enter_context(tc.tile_pool(name="gather", bufs=2))
        loc_pool = ctx.enter_context(tc.tile_pool(name="loc", bufs=2))
        loc1_pool = ctx.enter_context(tc.tile_pool(name="loc1", bufs=1))
        hbar_pool = ctx.enter_context(tc.tile_pool(name="hbar", bufs=1))
        out_pool = ctx.enter_context(tc.tile_pool(name="outstage", bufs=1))

        ps_score = ctx.enter_context(tc.tile_pool(name="ps_score", bufs=3, space="PSUM"))
        ps_qh = ctx.enter_context(tc.tile_pool(name="ps_qh", bufs=2, space="PSUM"))
        ps_out = ctx.enter_context(tc.tile_pool(name="ps_out", bufs=2, space="PSUM"))
        ps_small = ctx.enter_context(tc.tile_pool(name="ps_small", bufs=1, space="PSUM"))
        # group-head A/B matmuls share the qh pool's 2 banks (same tag)
        ps_ab = ps_qh

        # ---- resident constants ----
        # Small/early-needed tensors first, then muT streamed in column
        # blocks so slot-0 compute starts ~4us in instead of after a 40us
        # monolithic prologue (Tile range-tracks the block writes).
        whh_sb = consts.tile([P, 4, D], f16)
        whd_sb = consts.tile([P, 4, D], f16)
        wls_sb = consts.tile([P, 4, D], f16)
        wm1t_sb = consts.tile([P, 4, D], f16)
        wm1b_sb = consts.tile([P, 4, D], f16)
        wmo_sb = consts.tile([P, 4, D], f16)
        # qh weights only; the rest of the consts load after group-0's
        # strip DMAs so the first qh matmul can start ~10us in.
        for sb, dr in ((whh_sb, whh), (whd_sb, whd), (wls_sb, wls)):
            for di in range(4):
                nc.sync.dma_start(sb[:, di, :], dr[di])
        trimask_sb = consts.tile([P, 256], f32)
        bm1t_sb = consts.tile([P, 4], f32)
        e8_sb = consts.tile([P, P], f32)
        rep_sb = consts.tile([16, P], f32)

        def emit_late_consts():
            for sb, dr in ((wm1t_sb, wm1t), (wm1b_sb, wm1b), (wmo_sb, wmo)):
                for di in range(4):
                    nc.sync.dma_start(sb[:, di, :], dr[di])
            nc.sync.dma_start(trimask_sb[:], trimask[:])
            nc.sync.dma_start(bm1t_sb[:], bm1t[:])
            nc.sync.dma_start(e8_sb[:], e8sel[:])
            nc.sync.dma_start(rep_sb[:], repm[:])
        muT_h_sb = consts.tile([P, 4, 4 + N], f16)
        muT_ls_sb = consts.tile([P, 4, N], f16)
        MBLK = 1024

        def emit_mublock(b0):
            """One 1024-key column block of muT (h+ls, all 4 di).  Block 0 is
            emitted before the loop; later blocks are staggered into early
            iterations so group-0 strip DMAs aren't stuck behind 8.4MB."""
            for di in range(4):
                nc.sync.dma_start(muT_h_sb[:, di, 4 + b0:4 + b0 + MBLK],
                                  muT_h[di, :, 4 + b0:4 + b0 + MBLK])
                nc.sync.dma_start(muT_ls_sb[:, di, b0:b0 + MBLK],
                                  muT_ls[di, :, b0:b0 + MBLK])
        hbar = hbar_pool.tile([P, 4, NSLOT * P], f16)

        def emit_group_dma(grp):
            """Load the group's local strips (queued early for group 0 so the
            late consts don't delay the first qh matmuls)."""
            mlh = loc_pool.tile([P, 4, 4, 132], f16, tag="mlh")
            nc.sync.dma_start(
                mlh[:], muloc_h[:, :, grp * 528:(grp + 1) * 528]
                .rearrange("a p (t c) -> p a t c", c=132))
            mlls = loc1_pool.tile([P, 4, 4, 132], f16, tag="mlls")
            nc.sync.dma_start(
                mlls[:], muloc_ls[:, :, grp * 528:(grp + 1) * 528]
                .rearrange("a p (t c) -> p a t c", c=132))
            return mlh, mlls

        def emit_qh_part(grp, mlh, mlls):
            """Batched 3-pass qh (FD=512) for the group's 512 queries."""
            # mh/256 for qh pass 3 (DVE 4x: cheap, only needs the DMA)
            mh_d = mhd_pool.tile([P, 4, 4, P], f16, tag="mh_d")
            nc.vector.tensor_scalar_mul(mh_d[:], mlh[:, :, :, 4:132], 1.0 / ASC)

            # batched 3-pass qh for the group's 512 queries
            qhh = qh_pool.tile([P, 4, 512], f16, tag="qhh")
            qhl = qh_pool.tile([P, 4, 512], f16, tag="qhl")
            qhhd = qh_pool.tile([P, 4, 512], f16, tag="qhhd")
            for do in range(4):
                qp = ps_qh.tile([P, 512], f32, tag="qh")
                for di in range(4):
                    nc.tensor.matmul(
                        qp[:], whh_sb[:, di, do * P:(do + 1) * P],
                        mlh[:, di, :, 4:132], start=(di == 0), stop=False)
                for di in range(4):
                    nc.tensor.matmul(
                        qp[:], whd_sb[:, di, do * P:(do + 1) * P],
                        mlls[:, di, :, 4:132], start=False, stop=False)
                for di in range(4):
                    nc.tensor.matmul(
                        qp[:], wls_sb[:, di, do * P:(do + 1) * P],
                        mh_d[:, di, :, :], start=False, stop=(di == 3))
                nc.scalar.copy(qhh[:, do, :], qp[:])
                nc.vector.tensor_tensor(qhl[:, do, :], qp[:], qhh[:, do, :],
                                        op=OP.subtract)
            nc.vector.tensor_scalar_mul(qhhd[:], qhh[:], 1.0 / ASC)
            return (qhh, qhl, qhhd)

        def emit_local_part(grp, mlh):
            """Local branch: hbar[:, dh, grp*512:...]."""
            r0 = grp * 512
            for dh in range(4):
                a_ps = ps_ab.tile([P, 512], f32, tag="qh")
                for di in range(4):
                    nc.tensor.matmul(
                        a_ps[:], wm1t_sb[:, di, dh * P:(dh + 1) * P],
                        mlh[:, di, :, 4:132], start=(di == 0), stop=(di == 3))
                aP = loc_pool.tile([P, 512], f16, tag="aP")
                nc.scalar.activation(aP[:], a_ps[:], AF.Identity,
                                     bias=bm1t_sb[:, dh:dh + 1])
                bb = loc_pool.tile([P, 4, 132], f16, tag="bb")
                for half in range(2):
                    b_ps = ps_ab.tile([P, 2, 132], f32, tag="qh")
                    for di in range(4):
                        nc.tensor.matmul(
                            b_ps[:], wm1b_sb[:, di, dh * P:(dh + 1) * P],
                            mlh[:, di, 2 * half:2 * half + 2, :],
                            start=(di == 0), stop=(di == 3))
                    nc.scalar.copy(bb[:, 2 * half:2 * half + 2, :], b_ps[:])
                sil = loc1_pool.tile([P, 4, 512], f16, tag="sil")
                x4 = loc_pool.tile([P, 4, 512], f16, tag="xw", bufs=1)
                for w in range(1, WIN + 1):
                    nc.vector.tensor_tensor(
                        x4[:, w - 1, :], aP[:], bb[:, :, 4 - w:132 - w],
                        op=OP.add)
                nc.scalar.activation(sil[:], x4[:], AF.Silu)
                t1 = loc_pool.tile([P, 512], f16, tag="t1")
                nc.vector.tensor_tensor(t1[:], sil[:, 0, :], sil[:, 1, :],
                                        op=OP.add)
                nc.vector.tensor_tensor(t1[:], t1[:], sil[:, 2, :], op=OP.add)
                nc.vector.tensor_tensor(hbar[:, dh, r0:r0 + 512], t1[:],
                                        sil[:, 3, :], op=OP.add)

        def emit_scores(s, qh3, strip, c_lo, c_hi):
            """Scores strip chunks [c_lo, c_hi) for slot s, 3 fp16 passes.
            Chunks are emitted in sweeps of 2 sharing each stationary operand
            (12 LDWEIGHTS per sweep instead of 24) so matmuls pipeline
            back-to-back instead of paying the weight load per matmul."""
            KR = 256 * (s + 1)
            qhh, qhl, qhhd = qh3
            ti = s % 4
            q0 = ti * P
            nchunks = (KR + 511) // 512
            c_hi = min(c_hi, nchunks)
            for w0 in range(c_lo, c_hi, 2):
                cs = list(range(w0, min(w0 + 2, c_hi)))
                tiles = [ps_score.tile([P, 512], f32, tag="sps",
                                       name=f"sps{s}_{c}") for c in cs]
                passes = ((qhh, muT_h_sb, 4), (qhhd, muT_ls_sb, 0),
                          (qhl, muT_h_sb, 4))
                for pi, (stat, mov, off) in enumerate(passes):
                    for di in range(4):
                        for j, c in enumerate(cs):
                            k0 = c * 512
                            csz = min(512, KR - k0)
                            nc.tensor.matmul(
                                tiles[j][:, :csz], stat[:, di, q0:q0 + P],
                                mov[:, di, off + k0:off + k0 + csz],
                                start=(pi == 0 and di == 0),
                                stop=(pi == 2 and di == 3))
                for j, c in enumerate(cs):
                    k0 = c * 512
                    csz = min(512, KR - k0)
                    nc.scalar.copy(strip[:, k0:k0 + csz], tiles[j][:, :csz])

        def emit_select_head(s, strip):
            """Front-loaded DVE chain: mask + top8 + softmax (via tanh).
            Emitted at iteration head so it runs gather-free and the
            gather window lands in the PE-heavy scores phase."""
            KR = 256 * (s + 1)
            nc.vector.tensor_tensor(
                strip[:, KR - 256:KR], strip[:, KR - 256:KR], trimask_sb[:],
                op=OP.add)
            v8 = top_pool.tile([P, TOPK], f32, tag="v8")
            nc.vector.max(out=v8[:], in_=strip[:, :KR])
            i8 = top_pool.tile([P, TOPK], dt.uint32, tag="i8")
            nc.vector.max_index(out=i8[:], in_max=v8[:], in_values=strip[:, :KR])

            # softmax over the kept 8 via tanh (same ACT table set as silu):
            #   e^{x} = sig/(1-sig),  sig = (1+tanh(x/2))/2,  x = v - max
            nmh = top_pool.tile([P, 1], f32, tag="nmh")
            nc.vector.tensor_scalar_mul(nmh[:], v8[:, 0:1], -0.5)
            t8 = top_pool.tile([P, TOPK], f32, tag="t8")
            nc.scalar.activation(t8[:], v8[:], AF.Tanh, bias=nmh[:], scale=0.5)
            sig = top_pool.tile([P, TOPK], f32, tag="sig")
            nc.vector.tensor_scalar(sig[:], t8[:], 0.5, 0.5,
                                    op0=OP.mult, op1=OP.add)
            onem = top_pool.tile([P, TOPK], f32, tag="onem")
            nc.vector.tensor_scalar(onem[:], sig[:], -1.0, 1.0,
                                    op0=OP.mult, op1=OP.add)
            rcp = top_pool.tile([P, TOPK], f32, tag="rcp")
            nc.vector.reciprocal(rcp[:], onem[:])
            e8v = top_pool.tile([P, TOPK], f32, tag="e8v")
            nc.vector.tensor_tensor(e8v[:], sig[:], rcp[:], op=OP.mult)
            zsum = top_pool.tile([P, 1], f32, tag="zsum")
            nc.vector.tensor_reduce(zsum[:], e8v[:], axis=mybir.AxisListType.X,
                                    op=OP.add)
            zr = top_pool.tile([P, 1], f32, tag="zr")
            nc.vector.reciprocal(zr[:], zsum[:])
            p8 = top_pool.tile([P, TOPK], f32, tag="p8")
            nc.vector.tensor_scalar_mul(p8[:], e8v[:], zr[:])
            # uint32 -> fp32 for the index-wrap matmuls
            i8f = top_pool.tile([P, TOPK], f32, tag="i8f")
            nc.gpsimd.tensor_copy(i8f[:], i8[:])
            # diag(p8_k) tiles for the PE weighted sum (DVE; keeps the
            # scalar queue free for strip-chunk PSUM evacuation)
            diag = diag_pool.tile([P, TOPK, P], f16, tag="diag")
            for k in range(TOPK):
                nc.vector.tensor_scalar_mul(diag[:, k, :], e8_sb[:],
                                            p8[:, k:k + 1])
            return i8f, diag

        def emit_select_tail(s, i8f):
            """Index wrap (tiny PE matmuls) + gather launch."""
            iw1 = ps_small.tile([16, 64], f32, tag="small")
            iw1v = iw1[:].rearrange("p (c b) -> p b c", b=8)
            for pb in range(8):
                nc.tensor.matmul(iw1v[:, pb, :], e8_sb[:, 16 * pb:16 * pb + 16],
                                 i8f[:], start=True, stop=True,
                                 skip_group_check=True)
            iw1s = top_pool.tile([16, 64], f32, tag="iw1s")
            nc.scalar.copy(iw1s[:], iw1[:])
            iw2 = ps_small.tile([P, 64], f32, tag="small")
            nc.tensor.matmul(iw2[:], rep_sb[:], iw1s[:], start=True, stop=True)
            iw = top_pool.tile([P, 64], dt.int16, tag="iw")
            nc.vector.tensor_copy(iw[:], iw2[:])

            # gather the 8 pre-projected V rows per query (fp16, 1KB each)
            g = g_pool.tile([P, TOPK, D], f16, tag="g")
            nc.gpsimd.dma_gather(g[:], vkeys[:], iw[:], num_idxs=TOPK * P,
                                 num_idxs_reg=TOPK * P, elem_size=D)
            return g

        def emit_outwsum(s, g, diag):
            """out rows for slot s = hbar_slice.T @ Wmo + sum_k diag(p8_k)@g_k,
            all accumulated in one PSUM bank."""
            o_ps = ps_out.tile([P, D], f32, tag="out")
            for dm in range(4):
                nc.tensor.matmul(
                    o_ps[:], hbar[:, dm, s * P:(s + 1) * P],
                    wmo_sb[:, dm, :], start=(dm == 0), stop=False)
            for k in range(TOPK):
                nc.tensor.matmul(
                    o_ps[:], diag[:, k, :], g[:, k, :],
                    start=False, stop=(k == TOPK - 1))
            ost = out_pool.tile([P, D], f16, tag="ost")
            nc.scalar.copy(ost[:], o_ps[:])
            nc.sync.dma_start(outR[s * P:(s + 1) * P, :], ost[:])

        # ---- software-pipelined emission ----
        # iteration s:
        #   select_head(s-1)     DVE runs gather-free at iteration start
        #   scores(s) c0..1      PE dives straight into the big matmuls
        #   select_tail(s-1)     tiny PE wrap matmuls + gather launch; the
        #                        gather's SBUF-port contention window lands
        #                        in the PE-heavy rest of scores(s)
        #   scores(s) c2..       |  outwsum(s-2)  |  diag(s-1) (scalar tail)
        #   group prefetch (s%4==3)
        # big slots first within each group: more PE work early per group
        # (HAM warm) and the kernel tail drains on the smallest slot.
        order = [3, 2, 1, 0, 7, 6, 5, 4, 11, 10, 9, 8, 15, 14, 13, 12]
        st = {}
        g0 = emit_group_dma(0)
        emit_late_consts()           # local weights etc. AFTER group-0 strips
        emit_mublock(0)
        grpst = {0: emit_qh_part(0, *g0)}
        emit_local_part(0, g0[0])
        gdma = {}
        for p in range(NSLOT + 2):
            if p in (0, 3, 6):       # stream remaining key blocks early
                emit_mublock(MBLK * (p // 3 + 1))
            nxt = p // 4 + 1
            if p % 4 == 0 and nxt < 4:
                gdma[nxt] = emit_group_dma(nxt)
            if 1 <= p <= NSLOT:
                i8f, diag = emit_select_head(order[p - 1], st[p - 1]["strip"])
                st[p - 1]["diag"] = diag
            if p < NSLOT:
                strip = strip_pool.tile([P, N], f32, tag="strip")
                st[p] = {"strip": strip}
                emit_scores(order[p], grpst[p // 4], strip, 0, 2)
            if 1 <= p <= NSLOT:
                st[p - 1]["g"] = emit_select_tail(order[p - 1], i8f)
            if p % 4 == 1 and nxt < 4:
                # local branch of the NEXT group fills the PE early
                emit_local_part(nxt, gdma[nxt][0])
            if p < NSLOT:
                emit_scores(order[p], grpst[p // 4], strip, 2, 8)
            if p >= 2:
                emit_outwsum(order[p - 2], st[p - 2]["g"], st[p - 2]["diag"])
                del st[p - 2]
            if p % 4 == 3 and nxt < 4:
                grpst[nxt] = emit_qh_part(nxt, *gdma[nxt])

    nc.compile()
    _cache["nc"] = nc
    return nc


def _prep_core_inputs(c, mh_all, mls_all, v_all, consts):
    """Host-side sharding/layout for core c (fp16 hi + scaled-lo mu parts)."""
    f16 = np.float16
    b, h = c // 2, c % 2
    mh = mh_all[b]                                # [N, D] f16
    mls = mls_all[b]                              # [N, D] f16
    muT_h = np.zeros((D, 4 + N), f16)
    muT_h[:, 4:] = mh.T
    muT_ls = np.ascontiguousarray(mls.T)
    t_own = list(range(h, 32, 2))

    def strips(m):
        out = np.zeros((NSLOT * 132, D), f16)
        for i, t in enumerate(t_own):
            lo = 128 * t - 4
            src_lo = max(lo, 0)
            out[i * 132 + (src_lo - lo):(i + 1) * 132] = m[src_lo:128 * t + 128]
        return np.ascontiguousarray(out.T)        # [D, NSLOT*132]

    tm = np.zeros((P, 256), np.float32)
    j = np.arange(128)[None, :]
    p = np.arange(128)[:, None]
    if h == 0:
        tm[:, :128] = np.where(j <= p, 0.0, NEG)
        tm[:, 128:] = NEG
    else:
        tm[:, 128:] = np.where(j <= p, 0.0, NEG)
    return dict(
        muT_h=muT_h.reshape(4, P, 4 + N),
        muT_ls=muT_ls.reshape(4, P, N),
        muloc_h=strips(mh).reshape(4, P, NSLOT * 132),
        muloc_ls=strips(mls).reshape(4, P, NSLOT * 132),
        vkeys=v_all[b],
        trimask=tm,
        **consts,
    )


def prep_in_maps(inputs):
    f32, f16 = np.float32, np.float16
    mu = np.asarray(inputs["mu"], f32)
    Wq = np.asarray(inputs["Wq"], f32)
    bq = np.asarray(inputs["bq"], f32)
    Wk = np.asarray(inputs["Wk"], f32)
    Wv = np.asarray(inputs["Wv"], f32)
    bv = np.asarray(inputs["bv"], f32)
    Wm1 = np.asarray(inputs["Wm1"], f32)
    bm1 = np.asarray(inputs["bm1"], f32)
    Wm2 = np.asarray(inputs["Wm2"], f32)
    bm2 = np.asarray(inputs["bm2"], f32)
    Wo = np.asarray(inputs["Wo"], f32)
    bo = np.asarray(inputs["bo"], f32)
    assert not bq.any(), "bq != 0 unsupported (adds a per-key score term)"

    Wqks = (Wq @ Wk.T / math.sqrt(D)).astype(f32)
    Wmo = ((Wm2 @ Wo[:D]) / WIN).astype(f32)
    Wvo = (Wv @ Wo[D:]).astype(f32)
    bconst = (bo + bm2 @ Wo[:D] + bv @ Wo[D:]).astype(f32)

    Wh = Wqks.astype(f16)
    Wl_s = ((Wqks - Wh.astype(f32)) * ASC).astype(f16)
    Wh_d = (Wh.astype(f32) / ASC).astype(f16)
    consts = dict(
        whh=Wh.reshape(4, P, D),
        whd=Wh_d.reshape(4, P, D),
        wls=Wl_s.reshape(4, P, D),
        wm1t=Wm1[:D].astype(f16).reshape(4, P, D),
        wm1b=Wm1[D:].astype(f16).reshape(4, P, D),
        wmo=Wmo.astype(f16).reshape(4, P, D),
        bm1t=np.ascontiguousarray(bm1.reshape(4, P).T),
        e8sel=np.eye(P, dtype=f32),
        repm=np.ascontiguousarray(np.tile(np.eye(16, dtype=f32), (1, 8))),
    )
    mh_all = mu.astype(f16)                                   # [B, N, D]
    mls_all = ((mu - mh_all.astype(f32)) * ASC).astype(f16)
    v_all = (mu @ Wvo + bconst).astype(f16)                   # [B, N, D]
    return [_prep_core_inputs(c, mh_all, mls_all, v_all, consts)
            for c in range(NCORES)]


def assemble(core_outs):
    """core_outs: list of outR arrays [2048, D] per core -> full [B, N, D]."""
    out = np.empty((B, N, D), np.float32)
    for c in range(NCORES):
        b, h = c // 2, c % 2
        oc = np.asarray(core_outs[c]).astype(np.float32)
        for s, t in enumerate(range(h, 32, 2)):
            out[b, 128 * t:128 * t + 128] = oc[128 * s:128 * s + 128]
    return out


def kernel(**inputs):
    nc = _build_program()
    in_maps = prep_in_maps(inputs)

    import os
    from concourse.bass_utils import run_bass_kernel_spmd
    trace = bool(int(os.environ.get("LR_TRACE", "0")))
    res = run_bass_kernel_spmd(nc, in_maps, core_ids=list(range(NCORES)),
                               trace=trace)
    _cache["last_results"] = res
    return assemble([res.results[c]["outR"] for c in range(NCORES)])


# revision 21
# speedup vs baseline: 1.0033x; 1.0033x over previous
"""Trainium2 Bass kernel for nn_LocalRouter (sparse_attention).

Computation (reference semantics):
  local:  h_w = silu(mu_n @ Wm1_top + mu_{n-w} @ Wm1_bot + bm1), w=1..4
          local = mean_w(h_w) @ Wm2 + bm2
  global: scores = (mu @ Wq) @ (mu @ Wk)^T / sqrt(D), causal; top-8 -> softmax
          global = probs @ mu @ Wv + bv        (rows of probs sum to 1)
  out = concat([local, global]) @ Wo + bo

Algebraic refactors (host-side, exact in fp32):
  scores = (mu @ Wqks) @ mu^T, Wqks = Wq @ Wk^T / sqrt(D)
  out[n] = (hbar @ Wmo)[n] + sum_k p_k V[idx_k]
    hbar = sum_w silu(...), Wmo = (Wm2 @ Wo_top)/4
    V = mu @ (Wv @ Wo_bot) + bconst   (bconst folds in exactly: sum_k p_k = 1)
    bconst = bo + bm2 @ Wo_top + bv @ Wo_bot

Precision: scores need ~2^-20 accuracy (top-8 boundary flips swap whole V
rows). Computed as a 3-pass fp16 hi/lo split (1 cyc/row on the PE):
  mu = mh + ml_s/256,  qh = qhh + qhl          (fp16 pairs; lo parts scaled
  Wqks = Wh + Wl_s/256                          into fp16-normal range)
  qh     = mh@Wh + ml_s@(Wh/256) + (mh/256)@Wl_s      [fp32 PSUM accum]
  scores = qhh@mh + qhl@mh + (qhh/256)@ml_s

Engine plan (per core; v2 restructure of the 625us baseline):
  - qh computed per GROUP of 4 slots at FD=512 (was per-slot FD=128): the
    FD=128 matmuls were LDWEIGHTS-bound (~82us -> ~41us).
  - top-8 weighted sum moved off the DVE (was ~112us of 1x-mode tensor_scalar
    with per-partition scalars) onto the PE as 8 diag(p8_k) @ g_k matmuls
    accumulated into the same PSUM bank as the out-row projection.  diag
    matrices are built on the Scalar engine (activation Copy, per-partition
    scale AP reading eye(128)).
  - softmax exp replaced with tanh: e^x = sig/(1-sig), sig = (1+tanh(x/2))/2.
    Tanh lives in the same ACT table set as Silu, so the per-slot
    ACT_TABLE_LOAD churn (17 loads, ~45us scalar) disappears.
  - group g+1's DMA loads + qh + local branch are PREFETCHED at slot 4g+3 so
    the PE never waits on qh tiles at group boundaries.
  - out rows written f16 (host upcasts).

Top-8 indices are rewrapped for dma_gather on-chip with tiny selection
matmuls (iw[p, c*8+pb] = i8[16*pb + p%16, c]).

Sharding: core c -> batch b=c//2, half h=c%2 owns query tiles {t: t%2==h}
(interleaved for causal load balance). Key range per slot s is 256*(s+1)
(h-independent; the h-dependent diagonal lives in the trimask data).
"""

import math
import numpy as np

B, N, D = 4, 4096, 512
WIN, TOPK = 4, 8
P = 128
NCORES = 8
NSLOT = 16            # query tiles owned per core
NEG = -1.0e30
ASC = 256.0           # hi/lo split scale (2^8)

_cache = {}


def _build_program():
    if "nc" in _cache:
        return _cache["nc"]
    from contextlib import ExitStack
    import concourse.bass as bass
    import concourse.tile as tile
    import concourse.mybir as mybir
    from concourse import bacc

    dt = mybir.dt
    AF = mybir.ActivationFunctionType
    OP = mybir.AluOpType

    nc = bacc.Bacc(
        "TRN2",
        target_bir_lowering=False,
        debug=False,
        enable_asserts=False,
        num_devices=NCORES,
    )

    f32, f16 = dt.float32, dt.float16
    # ---- DRAM I/O (per-core data; program identical on all cores) ----
    muT_h = nc.dram_tensor("muT_h", [4, P, 4 + N], f16, kind="ExternalInput").ap()
    muT_ls = nc.dram_tensor("muT_ls", [4, P, N], f16, kind="ExternalInput").ap()
    muloc_h = nc.dram_tensor("muloc_h", [4, P, NSLOT * 132], f16,
                             kind="ExternalInput").ap()
    muloc_ls = nc.dram_tensor("muloc_ls", [4, P, NSLOT * 132], f16,
                              kind="ExternalInput").ap()
    vkeys = nc.dram_tensor("vkeys", [N, D], f16, kind="ExternalInput").ap()
    whh = nc.dram_tensor("whh", [4, P, D], f16, kind="ExternalInput").ap()
    whd = nc.dram_tensor("whd", [4, P, D], f16, kind="ExternalInput").ap()
    wls = nc.dram_tensor("wls", [4, P, D], f16, kind="ExternalInput").ap()
    wm1t = nc.dram_tensor("wm1t", [4, P, D], f16, kind="ExternalInput").ap()
    wm1b = nc.dram_tensor("wm1b", [4, P, D], f16, kind="ExternalInput").ap()
    wmo = nc.dram_tensor("wmo", [4, P, D], f16, kind="ExternalInput").ap()
    trimask = nc.dram_tensor("trimask", [P, 256], f32, kind="ExternalInput").ap()
    bm1t = nc.dram_tensor("bm1t", [P, 4], f32, kind="ExternalInput").ap()
    e8sel = nc.dram_tensor("e8sel", [P, P], f32, kind="ExternalInput").ap()
    repm = nc.dram_tensor("repm", [16, P], f32, kind="ExternalInput").ap()
    outR = nc.dram_tensor("outR", [NSLOT * P, D], f16, kind="ExternalOutput").ap()

    with tile.TileContext(nc) as tc, ExitStack() as ctx:
        consts = ctx.enter_context(tc.tile_pool(name="consts", bufs=1))
        qh_pool = ctx.enter_context(tc.tile_pool(name="qh", bufs=1))
        mhd_pool = ctx.enter_context(tc.tile_pool(name="mhd", bufs=1))
        strip_pool = ctx.enter_context(tc.tile_pool(name="strip", bufs=2))
        top_pool = ctx.enter_context(tc.tile_pool(name="top", bufs=2))
        diag_pool = ctx.enter_context(tc.tile_pool(name="diag", bufs=2))
        g_pool = ctx.enter_context(tc.tile_pool(name="gather", bufs=2))
        loc_pool = ctx.enter_context(tc.tile_pool(name="loc", bufs=2))
        loc1_pool = ctx.enter_context(tc.tile_pool(name="loc1", bufs=1))
        hbar_pool = ctx.enter_context(tc.tile_pool(name="hbar", bufs=1))
        out_pool = ctx.enter_context(tc.tile_pool(name="outstage", bufs=1))

        ps_score = ctx.enter_context(tc.tile_pool(name="ps_score", bufs=3, space="PSUM"))
        ps_qh = ctx.enter_context(tc.tile_pool(name="ps_qh", bufs=2, space="PSUM"))
        ps_out = ctx.enter_context(tc.tile_pool(name="ps_out", bufs=2, space="PSUM"))
        ps_small = ctx.enter_context(tc.tile_pool(name="ps_small", bufs=1, space="PSUM"))
        # group-head A/B matmuls share the qh pool's 2 banks (same tag)
        ps_ab = ps_qh

        # ---- resident constants ----
        # Small/early-needed tensors first, then muT streamed in column
        # blocks so slot-0 compute starts ~4us in instead of after a 40us
        # monolithic prologue (Tile range-tracks the block writes).
        whh_sb = consts.tile([P, 4, D], f16)
        whd_sb = consts.tile([P, 4, D], f16)
        wls_sb = consts.tile([P, 4, D], f16)
        wm1t_sb = consts.tile([P, 4, D], f16)
        wm1b_sb = consts.tile([P, 4, D], f16)
        wmo_sb = consts.tile([P, 4, D], f16)
        # qh weights only; the rest of the consts load after group-0's
        # strip DMAs so the first qh matmul can start ~10us in.
        for sb, dr in ((whh_sb, whh), (whd_sb, whd), (wls_sb, wls)):
            for di in range(4):
                nc.sync.dma_start(sb[:, di, :], dr[di])
        trimask_sb = consts.tile([P, 256], f32)
        bm1t_sb = consts.tile([P, 4], f32)
        e8_sb = consts.tile([P, P], f32)
        rep_sb = consts.tile([16, P], f32)

        def emit_late_consts():
            for sb, dr in ((wm1t_sb, wm1t), (wm1b_sb, wm1b), (wmo_sb, wmo)):
                for di in range(4):
                    nc.sync.dma_start(sb[:, di, :], dr[di])
            nc.sync.dma_start(trimask_sb[:], trimask[:])
            nc.sync.dma_start(bm1t_sb[:], bm1t[:])
            nc.sync.dma_start(e8_sb[:], e8sel[:])
            nc.sync.dma_start(rep_sb[:], repm[:])
        muT_h_sb = consts.tile([P, 4, 4 + N], f16)
        muT_ls_sb = consts.tile([P, 4, N], f16)
        MBLK = 1024

        def emit_mublock(b0):
            """One 1024-key column block of muT (h+ls, all 4 di).  Block 0 is
            emitted before the loop; later blocks are staggered into early
            iterations so group-0 strip DMAs aren't stuck behind 8.4MB."""
            for di in range(4):
                nc.sync.dma_start(muT_h_sb[:, di, 4 + b0:4 + b0 + MBLK],
                                  muT_h[di, :, 4 + b0:4 + b0 + MBLK])
                nc.sync.dma_start(muT_ls_sb[:, di, b0:b0 + MBLK],
                                  muT_ls[di, :, b0:b0 + MBLK])
        hbar = hbar_pool.tile([P, 4, NSLOT * P], f16)

        def emit_group_dma(grp):
            """Load the group's local strips (queued early for group 0 so the
            late consts don't delay the first qh matmuls)."""
            mlh = loc_pool.tile([P, 4, 4, 132], f16, tag="mlh")
            nc.sync.dma_start(
                mlh[:], muloc_h[:, :, grp * 528:(grp + 1) * 528]
                .rearrange("a p (t c) -> p a t c", c=132))
            mlls = loc1_pool.tile([P, 4, 4, 132], f16, tag="mlls")
            nc.sync.dma_start(
                mlls[:], muloc_ls[:, :, grp * 528:(grp + 1) * 528]
                .rearrange("a p (t c) -> p a t c", c=132))
            return mlh, mlls

        def emit_qh_part(grp, mlh, mlls):
            """Batched 3-pass qh (FD=512) for the group's 512 queries."""
            # mh/256 for qh pass 3 (DVE 4x: cheap, only needs the DMA)
            mh_d = mhd_pool.tile([P, 4, 4, P], f16, tag="mh_d")
            nc.vector.tensor_scalar_mul(mh_d[:], mlh[:, :, :, 4:132], 1.0 / ASC)

            # batched 3-pass qh for the group's 512 queries
            qhh = qh_pool.tile([P, 4, 512], f16, tag="qhh")
            qhl = qh_pool.tile([P, 4, 512], f16, tag="qhl")
            qhhd = qh_pool.tile([P, 4, 512], f16, tag="qhhd")
            for do in range(4):
                qp = ps_qh.tile([P, 512], f32, tag="qh")
                for di in range(4):
                    nc.tensor.matmul(
                        qp[:], whh_sb[:, di, do * P:(do + 1) * P],
                        mlh[:, di, :, 4:132], start=(di == 0), stop=False)
                for di in range(4):
                    nc.tensor.matmul(
                        qp[:], whd_sb[:, di, do * P:(do + 1) * P],
                        mlls[:, di, :, 4:132], start=False, stop=False)
                for di in range(4):
                    nc.tensor.matmul(
                        qp[:], wls_sb[:, di, do * P:(do + 1) * P],
                        mh_d[:, di, :, :], start=False, stop=(di == 3))
                nc.scalar.copy(qhh[:, do, :], qp[:])
                nc.vector.tensor_tensor(qhl[:, do, :], qp[:], qhh[:, do, :],
                                        op=OP.subtract)
            nc.vector.tensor_scalar_mul(qhhd[:], qhh[:], 1.0 / ASC)
            return (qhh, qhl, qhhd)

        def emit_local_part(grp, mlh):
            """Local branch: hbar[:, dh, grp*512:...]."""
            r0 = grp * 512
            for dh in range(4):
                a_ps = ps_ab.tile([P, 512], f32, tag="qh")
                for di in range(4):
                    nc.tensor.matmul(
                        a_ps[:], wm1t_sb[:, di, dh * P:(dh + 1) * P],
                        mlh[:, di, :, 4:132], start=(di == 0), stop=(di == 3))
                aP = loc_pool.tile([P, 512], f16, tag="aP")
                nc.scalar.activation(aP[:], a_ps[:], AF.Identity,
                                     bias=bm1t_sb[:, dh:dh + 1])
                bb = loc_pool.tile([P, 4, 132], f16, tag="bb")
                for half in range(2):
                    b_ps = ps_ab.tile([P, 2, 132], f32, tag="qh")
                    for di in range(4):
                        nc.tensor.matmul(
                            b_ps[:], wm1b_sb[:, di, dh * P:(dh + 1) * P],
                            mlh[:, di, 2 * half:2 * half + 2, :],
                            start=(di == 0), stop=(di == 3))
                    nc.scalar.copy(bb[:, 2 * half:2 * half + 2, :], b_ps[:])
                sil = loc1_pool.tile([P, 4, 512], f16, tag="sil")
                x4 = loc_pool.tile([P, 4, 512], f16, tag="xw", bufs=1)
                for w in range(1, WIN + 1):
                    nc.vector.tensor_tensor(
                        x4[:, w - 1, :], aP[:], bb[:, :, 4 - w:132 - w],
                        op=OP.add)
                nc.scalar.activation(sil[:], x4[:], AF.Silu)
                t1 = loc_pool.tile([P, 512], f16, tag="t1")
                nc.vector.tensor_tensor(t1[:], sil[:, 0, :], sil[:, 1, :],
                                        op=OP.add)
                nc.vector.tensor_tensor(t1[:], t1[:], sil[:, 2, :], op=OP.add)
                nc.vector.tensor_tensor(hbar[:, dh, r0:r0 + 512], t1[:],
                                        sil[:, 3, :], op=OP.add)

        def emit_scores(s, qh3, strip, c_lo, c_hi):
            """Scores strip chunks [c_lo, c_hi) for slot s, 3 fp16 passes.
            Chunks are emitted in sweeps of 2 sharing each stationary operand
            (12 LDWEIGHTS per sweep instead of 24) so matmuls pipeline
            back-to-back instead of paying the weight load per matmul."""
            KR = 256 * (s + 1)
            qhh, qhl, qhhd = qh3
            ti = s % 4
            q0 = ti * P
            nchunks = (KR + 511) // 512
            c_hi = min(c_hi, nchunks)
            for w0 in range(c_lo, c_hi, 2):
                cs = list(range(w0, min(w0 + 2, c_hi)))
                tiles = [ps_score.tile([P, 512], f32, tag="sps",
                                       name=f"sps{s}_{c}") for c in cs]
                passes = ((qhh, muT_h_sb, 4), (qhhd, muT_ls_sb, 0),
                          (qhl, muT_h_sb, 4))
                for pi, (stat, mov, off) in enumerate(passes):
                    for di in range(4):
                        for j, c in enumerate(cs):
                            k0 = c * 512
                            csz = min(512, KR - k0)
                            nc.tensor.matmul(
                                tiles[j][:, :csz], stat[:, di, q0:q0 + P],
                                mov[:, di, off + k0:off + k0 + csz],
                                start=(pi == 0 and di == 0),
                                stop=(pi == 2 and di == 3))
                for j, c in enumerate(cs):
                    k0 = c * 512
                    csz = min(512, KR - k0)
                    nc.scalar.copy(strip[:, k0:k0 + csz], tiles[j][:, :csz])

        def emit_select_head(s, strip):
            """Front-loaded DVE chain: mask + top8 + softmax (via tanh).
            Emitted at iteration head so it runs gather-free and the
            gather window lands in the PE-heavy scores phase."""
            KR = 256 * (s + 1)
            nc.vector.tensor_tensor(
                strip[:, KR - 256:KR], strip[:, KR - 256:KR], trimask_sb[:],
                op=OP.add)
            v8 = top_pool.tile([P, TOPK], f32, tag="v8")
            nc.vector.max(out=v8[:], in_=strip[:, :KR])
            i8 = top_pool.tile([P, TOPK], dt.uint32, tag="i8")
            nc.vector.max_index(out=i8[:], in_max=v8[:], in_values=strip[:, :KR])

            # softmax over the kept 8 via tanh (same ACT table set as silu):
            #   e^{x} = sig/(1-sig),  sig = (1+tanh(x/2))/2,  x = v - max
            nmh = top_pool.tile([P, 1], f32, tag="nmh")
            nc.vector.tensor_scalar_mul(nmh[:], v8[:, 0:1], -0.5)
            t8 = top_pool.tile([P, TOPK], f32, tag="t8")
            nc.scalar.activation(t8[:], v8[:], AF.Tanh, bias=nmh[:], scale=0.5)
            sig = top_pool.tile([P, TOPK], f32, tag="sig")
            nc.vector.tensor_scalar(sig[:], t8[:], 0.5, 0.5,
                                    op0=OP.mult, op1=OP.add)
            onem = top_pool.tile([P, TOPK], f32, tag="onem")
            nc.vector.tensor_scalar(onem[:], sig[:], -1.0, 1.0,
                                    op0=OP.mult, op1=OP.add)
            rcp = top_pool.tile([P, TOPK], f32, tag="rcp")
            nc.vector.reciprocal(rcp[:], onem[:])
            e8v = top_pool.tile([P, TOPK], f32, tag="e8v")
            nc.vector.tensor_tensor(e8v[:], sig[:], rcp[:], op=OP.mult)
            zsum = top_pool.tile([P, 1], f32, tag="zsum")
            nc.vector.tensor_reduce(zsum[:], e8v[:], axis=mybir.AxisListType.X,
                                    op=OP.add)
            zr = top_pool.tile([P, 1], f32, tag="zr")
            nc.vector.reciprocal(zr[:], zsum[:])
            p8 = top_pool.tile([P, TOPK], f32, tag="p8")
            nc.vector.tensor_scalar_mul(p8[:], e8v[:], zr[:])
            # uint32 -> fp32 for the index-wrap matmuls
            i8f = top_pool.tile([P, TOPK], f32, tag="i8f")
            nc.gpsimd.tensor_copy(i8f[:], i8[:])
            # diag(p8_k) tiles for the PE weighted sum (DVE; keeps the
            # scalar queue free for strip-chunk PSUM evacuation)
            diag = diag_pool.tile([P, TOPK, P], f16, tag="diag")
            for k in range(TOPK):
                nc.vector.tensor_scalar_mul(diag[:, k, :], e8_sb[:],
                                            p8[:, k:k + 1])
            return i8f, diag

        def emit_select_tail(s, i8f):
            """Index wrap (tiny PE matmuls) + gather launch."""
            iw1 = ps_small.tile([16, 64], f32, tag="small")
            iw1v = iw1[:].rearrange("p (c b) -> p b c", b=8)
            for pb in range(8):
                nc.tensor.matmul(iw1v[:, pb, :], e8_sb[:, 16 * pb:16 * pb + 16],
                                 i8f[:], start=True, stop=True,
                                 skip_group_check=True)
            iw1s = top_pool.tile([16, 64], f32, tag="iw1s")
            nc.scalar.copy(iw1s[:], iw1[:])
            iw2 = ps_small.tile([P, 64], f32, tag="small")
            nc.tensor.matmul(iw2[:], rep_sb[:], iw1s[:], start=True, stop=True)
            iw = top_pool.tile([P, 64], dt.int16, tag="iw")
            nc.vector.tensor_copy(iw[:], iw2[:])

            # gather the 8 pre-projected V rows per query (fp16, 1KB each)
            g = g_pool.tile([P, TOPK, D], f16, tag="g")
            nc.gpsimd.dma_gather(g[:], vkeys[:], iw[:], num_idxs=TOPK * P,
                                 num_idxs_reg=TOPK * P, elem_size=D)
            return g

        def emit_outwsum(s, g, diag):
            """out rows for slot s = hbar_slice.T @ Wmo + sum_k diag(p8_k)@g_k,
            all accumulated in one PSUM bank."""
            o_ps = ps_out.tile([P, D], f32, tag="out")
            for dm in range(4):
                nc.tensor.matmul(
                    o_ps[:], hbar[:, dm, s * P:(s + 1) * P],
                    wmo_sb[:, dm, :], start=(dm == 0), stop=False)
            for k in range(TOPK):
                nc.tensor.matmul(
                    o_ps[:], diag[:, k, :], g[:, k, :],
                    start=False, stop=(k == TOPK - 1))
            ost = out_pool.tile([P, D], f16, tag="ost")
            nc.scalar.copy(ost[:], o_ps[:])
            nc.sync.dma_start(outR[s * P:(s + 1) * P, :], ost[:])

        # ---- software-pipelined emission ----
        # iteration s:
        #   select_head(s-1)     DVE runs gather-free at iteration start
        #   scores(s) c0..1      PE dives straight into the big matmuls
        #   select_tail(s-1)     tiny PE wrap matmuls + gather launch; the
        #                        gather's SBUF-port contention window lands
        #                        in the PE-heavy rest of scores(s)
        #   scores(s) c2..       |  outwsum(s-2)  |  diag(s-1) (scalar tail)
        #   group prefetch (s%4==3)
        # group order 0,3,2,1 with big slots first within each group: big
        # slots run mid-pipeline and the kernel tail drains on small slot 4.
        grp_order = [0, 3, 2, 1]
        order = [s for g in grp_order for s in range(4 * g + 3, 4 * g - 1, -1)]
        st = {}
        g0 = emit_group_dma(0)
        emit_late_consts()           # local weights etc. AFTER group-0 strips
        emit_mublock(0)
        grpst = {0: emit_qh_part(0, *g0)}
        emit_local_part(0, g0[0])
        gdma = {}
        for p in range(NSLOT + 2):
            if p in (0, 1, 2):       # group 3 runs next: all key blocks early
                emit_mublock(MBLK * (p + 1))
            nxt = grp_order[p // 4 + 1] if p // 4 + 1 < 4 else None
            if p % 4 == 0 and nxt is not None:
                gdma[nxt] = emit_group_dma(nxt)
            if 1 <= p <= NSLOT:
                i8f, diag = emit_select_head(order[p - 1], st[p - 1]["strip"])
                st[p - 1]["diag"] = diag
            if p < NSLOT:
                strip = strip_pool.tile([P, N], f32, tag="strip")
                st[p] = {"strip": strip}
                emit_scores(order[p], grpst[order[p] // 4], strip, 0, 2)
            if 1 <= p <= NSLOT:
                st[p - 1]["g"] = emit_select_tail(order[p - 1], i8f)
            if p % 4 == 1 and nxt is not None:
                # local branch of the NEXT group fills the PE early
                emit_local_part(nxt, gdma[nxt][0])
            if p < NSLOT:
                emit_scores(order[p], grpst[order[p] // 4], strip, 2, 8)
            if p >= 2:
                emit_outwsum(order[p - 2], st[p - 2]["g"], st[p - 2]["diag"])
                del st[p - 2]
            if p % 4 == 3 and nxt is not None:
                grpst[nxt] = emit_qh_part(nxt, *gdma[nxt])

    nc.compile()
    _cache["nc"] = nc
    return nc


def _prep_core_inputs(c, mh_all, mls_all, v_all, consts):
    """Host-side sharding/layout for core c (fp16 hi + scaled-lo mu parts)."""
    f16 = np.float16
    b, h = c // 2, c % 2
    mh = mh_all[b]                                # [N, D] f16
    mls = mls_all[b]                              # [N, D] f16
    muT_h = np.zeros((D, 4 + N), f16)
    muT_h[:, 4:] = mh.T
    muT_ls = np.ascontiguousarray(mls.T)
    t_own = list(range(h, 32, 2))

    def strips(m):
        out = np.zeros((NSLOT * 132, D), f16)
        for i, t in enumerate(t_own):
            lo = 128 * t - 4
            src_lo = max(lo, 0)
            out[i * 132 + (src_lo - lo):(i + 1) * 132] = m[src_lo:128 * t + 128]
        return np.ascontiguousarray(out.T)        # [D, NSLOT*132]

    tm = np.zeros((P, 256), np.float32)
    j = np.arange(128)[None, :]
    p = np.arange(128)[:, None]
    if h == 0:
        tm[:, :128] = np.where(j <= p, 0.0, NEG)
        tm[:, 128:] = NEG
    else:
        tm[:, 128:] = np.where(j <= p, 0.0, NEG)
    return dict(
        muT_h=muT_h.reshape(4, P, 4 + N),
        muT_ls=muT_ls.reshape(4, P, N),
        muloc_h=strips(mh).reshape(4, P, NSLOT * 132),
        muloc_ls=strips(mls).reshape(4, P, NSLOT * 132),
        vkeys=v_all[b],
        trimask=tm,
        **consts,
    )


def prep_in_maps(inputs):
    f32, f16 = np.float32, np.float16
    mu = np.asarray(inputs["mu"], f32)
    Wq = np.asarray(inputs["Wq"], f32)
    bq = np.asarray(inputs["bq"], f32)
    Wk = np.asarray(inputs["Wk"], f32)
    Wv = np.asarray(inputs["Wv"], f32)
    bv = np.asarray(inputs["bv"], f32)
    Wm1 = np.asarray(inputs["Wm1"], f32)
    bm1 = np.asarray(inputs["bm1"], f32)
    Wm2 = np.asarray(inputs["Wm2"], f32)
    bm2 = np.asarray(inputs["bm2"], f32)
    Wo = np.asarray(inputs["Wo"], f32)
    bo = np.asarray(inputs["bo"], f32)
    assert not bq.any(), "bq != 0 unsupported (adds a per-key score term)"

    Wqks = (Wq @ Wk.T / math.sqrt(D)).astype(f32)
    Wmo = ((Wm2 @ Wo[:D]) / WIN).astype(f32)
    Wvo = (Wv @ Wo[D:]).astype(f32)
    bconst = (bo + bm2 @ Wo[:D] + bv @ Wo[D:]).astype(f32)

    Wh = Wqks.astype(f16)
    Wl_s = ((Wqks - Wh.astype(f32)) * ASC).astype(f16)
    Wh_d = (Wh.astype(f32) / ASC).astype(f16)
    consts = dict(
        whh=Wh.reshape(4, P, D),
        whd=Wh_d.reshape(4, P, D),
        wls=Wl_s.reshape(4, P, D),
        wm1t=Wm1[:D].astype(f16).reshape(4, P, D),
        wm1b=Wm1[D:].astype(f16).reshape(4, P, D),
        wmo=Wmo.astype(f16).reshape(4, P, D),
        bm1t=np.ascontiguousarray(bm1.reshape(4, P).T),
        e8sel=np.eye(P, dtype=f32),
        repm=np.ascontiguousarray(np.tile(np.eye(16, dtype=f32), (1, 8))),
    )
    mh_all = mu.astype(f16)                                   # [B, N, D]
    mls_all = ((mu - mh_all.astype(f32)) * ASC).astype(f16)
    v_all = (mu @ Wvo + bconst).astype(f16)                   # [B, N, D]
    return [_prep_core_inputs(c, mh_all, mls_all, v_all, consts)
            for c in range(NCORES)]


def assemble(core_outs):
    """core_outs: list of outR arrays [2048, D] per core -> full [B, N, D]."""
    out = np.empty((B, N, D), np.float32)
    for c in range(NCORES):
        b, h = c // 2, c % 2
        oc = np.asarray(core_outs[c]).astype(np.float32)
        for s, t in enumerate(range(h, 32, 2)):
            out[b, 128 * t:128 * t + 128] = oc[128 * s:128 * s + 128]
    return out


def kernel(**inputs):
    nc = _build_program()
    in_maps = prep_in_maps(inputs)

    import os
    from concourse.bass_utils import run_bass_kernel_spmd
    trace = bool(int(os.environ.get("LR_TRACE", "0")))
    res = run_bass_kernel_spmd(nc, in_maps, core_ids=list(range(NCORES)),
                               trace=trace)
    _cache["last_results"] = res
    return assemble([res.results[c]["outR"] for c in range(NCORES)])
